# revision 6
# baseline (speedup 1.0000x reference)
"""Fake-quantized segmented linear (DefaultSegmentLinear) on 8 TRN2 NeuronCores.

out = x_fq @ w_fq.T + bias  where
  w_fq = per-tensor symmetric int8 fake quant of weight
  x_fq = per-chunk (4 chunks along in_features) symmetric int8 fake quant of x

Strategy: data-parallel over tokens (B*S=16384 -> 2048/core).  Quantized
values are integers in [-127,127], exactly representable in bf16, so the
matmul runs on the tensor engine in bf16 with fp32 PSUM accumulation and
the chunk scales are applied by rescaling PSUM in place between chunk
accumulation groups (exact up to a few fp32 roundings).
"""

import numpy as np

import concourse.bass as bass
import concourse.bass_isa as bass_isa
import concourse.mybir as mybir
import concourse.tile as tile
from concourse import bacc
from concourse.bass import ds, ts
from concourse.bass_utils import run_bass_kernel_spmd
from concourse.masks import make_identity

FP32 = mybir.dt.float32
BF16 = mybir.dt.bfloat16
AX = mybir.AxisListType
ALU = mybir.AluOpType
AF = mybir.ActivationFunctionType

QMAX = 127.0
EPS = 1e-8
MAGIC = 1.5 * (2.0**23)  # fp32 add of this rounds to nearest-even integer

# full problem dims
N_CORES = 8
B_, S_, F_, O_ = 4, 4096, 4096, 4096
CHUNKS = 4


def build_nc(T, F, O, n_cores, grp=4, op_w=512):
    """Build the per-core SPMD Bass program.

    T: tokens per core; F: in features; O: out features.
    grp: out-tiles interleaved per PSUM group (rescales overlap matmuls).
    op_w: output-panel width (N of each matmul, <= 512).
    """
    C = CHUNKS
    D = F // C
    KT = F // 128          # k tiles
    KPC = KT // C          # k tiles per chunk
    TT = T // 128          # token tiles
    NP = O // op_w         # output panels
    WSL = O // n_cores     # weight-slice rows per core (for distributed amax)
    assert F % (128 * C) == 0 and T % 128 == 0 and O % op_w == 0
    assert WSL % 128 == 0
    grp = min(grp, TT)
    assert TT % grp == 0

    nc = bacc.Bacc(
        "TRN2", target_bir_lowering=False, debug=False, num_devices=n_cores
    )

    x = nc.dram_tensor("x", [T, F], FP32, kind="ExternalInput")
    w = nc.dram_tensor("w", [O, F], FP32, kind="ExternalInput")
    wsl = nc.dram_tensor("wsl", [WSL, F], FP32, kind="ExternalInput")
    b = nc.dram_tensor("b", [1, O], FP32, kind="ExternalInput")
    out = nc.dram_tensor("out", [T, O], FP32, kind="ExternalOutput")
    wq_nat = nc.dram_tensor("wq_nat", [O, F], BF16)
    cc_in = nc.dram_tensor("cc_in", [1, 8], FP32)
    cc_out = nc.dram_tensor("cc_out", [1, 8], FP32)

    with tile.TileContext(nc) as tc:
        with tc.tile_pool(name="persist", bufs=1) as persist:
            # resident transposed quantized x: [128 f-part, KT, T] bf16
            xqT = persist.tile([128, KT, T], BF16, tag="xqT")
            gb = persist.tile([128, 8], FP32, tag="gb")        # global amaxes
            s5 = persist.tile([128, 8], FP32, tag="s5")        # scales
            inv5 = persist.tile([128, 8], FP32, tag="inv5")    # 1/scales
            rr = persist.tile([128, 4], FP32, tag="rr")        # r01,r12,r23, s3*sw
            binv = persist.tile([128, 1], FP32, tag="binv")    # 1/(sw*s0)
            ones = persist.tile([1, 128], FP32, tag="ones")
            ident = persist.tile([128, 128], BF16, tag="ident")

            nc.gpsimd.memset(ones[:, :], 1.0)
            make_identity(nc, ident[:, :])

            # ---------------- Phase A: amax ----------------
            with (
                tc.tile_pool(name="ldA", bufs=3) as ldA,
                tc.tile_pool(name="partA", bufs=1) as partA,
            ):
                xpart = partA.tile([128, TT, C], FP32, tag="xpart")
                wpart = partA.tile([128, WSL // 128], FP32, tag="wpart")
                amax5 = partA.tile([128, 8], FP32, tag="amax5")
                amaxr = partA.tile([128, 8], FP32, tag="amaxr")
                g1 = partA.tile([1, 8], FP32, tag="g1")

                for i in range(TT):
                    t = ldA.tile([128, F], FP32, tag="ld")
                    nc.sync.dma_start(t[:, :], x[ts(i, 128), :])
                    nc.vector.tensor_reduce(
                        out=xpart[:, i, :],
                        in_=t.rearrange("p (c d) -> p c d", c=C),
                        axis=AX.X,
                        op=ALU.max,
                        apply_absolute_value=True,
                    )
                for j in range(WSL // 128):
                    t = ldA.tile([128, F], FP32, tag="ld")
                    nc.sync.dma_start(t[:, :], wsl[ts(j, 128), :])
                    nc.vector.tensor_reduce(
                        out=wpart[:, j : j + 1],
                        in_=t[:, :],
                        axis=AX.X,
                        op=ALU.max,
                        apply_absolute_value=True,
                    )
                nc.gpsimd.memset(amax5[:, :], 0.0)
                nc.vector.tensor_reduce(
                    out=amax5[:, 0:C],
                    in_=xpart.rearrange("p i c -> p c i"),
                    axis=AX.X,
                    op=ALU.max,
                )
                nc.vector.tensor_reduce(
                    out=amax5[:, C : C + 1],
                    in_=wpart[:, :],
                    axis=AX.X,
                    op=ALU.max,
                )
                nc.gpsimd.partition_all_reduce(
                    amaxr[:, :], amax5[:, :], channels=128,
                    reduce_op=bass_isa.ReduceOp.max,
                )
                # 8-core max-allreduce of the 5 amaxes via DRAM bounce
                nc.sync.dma_start(cc_in[:, :], amaxr[0:1, :])
                nc.gpsimd.collective_compute(
                    "AllReduce",
                    ALU.max,
                    replica_groups=[list(range(n_cores))],
                    ins=[cc_in[:, :].opt()],
                    outs=[cc_out[:, :].opt()],
                )
                nc.sync.dma_start(g1[:, :], cc_out[:, :])
                nc.gpsimd.partition_broadcast(gb[:, :], g1[:, :], channels=128)

            # scales: s = max(amax * (1/127), eps)   (~1ulp vs exact divide)
            nc.vector.tensor_scalar(
                out=s5[:, 0 : C + 1], in0=gb[:, 0 : C + 1],
                scalar1=1.0 / QMAX, scalar2=EPS, op0=ALU.mult, op1=ALU.max,
            )
            nc.vector.reciprocal(inv5[:, 0 : C + 1], s5[:, 0 : C + 1])
            # chunk ratios r_c = s_c * (1/s_{c+1}); final scale = s3 * sw
            nc.vector.tensor_tensor(
                out=rr[:, 0 : C - 1], in0=s5[:, 0 : C - 1], in1=inv5[:, 1:C],
                op=ALU.mult,
            )
            nc.vector.tensor_tensor(
                out=rr[:, C - 1 : C], in0=s5[:, C - 1 : C], in1=s5[:, C : C + 1],
                op=ALU.mult,
            )
            nc.vector.tensor_tensor(
                out=binv[:, :], in0=inv5[:, C : C + 1], in1=inv5[:, 0:1],
                op=ALU.mult,
            )

            # ---------------- Phase B: quantize ----------------
            with (
                tc.tile_pool(name="ldB", bufs=2) as ldB,
                tc.tile_pool(name="stg", bufs=2) as stg,
                tc.tile_pool(name="qtmp", bufs=2) as qtmp,
                tc.tile_pool(name="tpsum", bufs=4, space="PSUM") as tpsum,
            ):
                # B1: x -> integer bf16, transposed on-chip via PE
                for i in range(TT):
                    xt = ldB.tile([128, F], FP32, tag="ld")
                    nc.sync.dma_start(xt[:, :], x[ts(i, 128), :])
                    xqn = stg.tile([128, F], BF16, tag="stg")
                    for c in range(C):
                        t1 = qtmp.tile([128, D], FP32, tag="qt")
                        nc.vector.tensor_scalar(
                            out=t1[:, :], in0=xt[:, ts(c, D)],
                            scalar1=inv5[:, c : c + 1], scalar2=MAGIC,
                            op0=ALU.mult, op1=ALU.add,
                        )
                        nc.vector.tensor_scalar_add(
                            out=xqn[:, ts(c, D)], in0=t1[:, :], scalar1=-MAGIC,
                        )
                    for k in range(KT):
                        pt = tpsum.tile([128, 128], BF16, tag="tp")
                        nc.tensor.transpose(pt[:, :], xqn[:, ts(k, 128)], ident[:, :])
                        nc.vector.tensor_copy(xqT[:, k, ts(i, 128)], pt[:, :])

                # B2: w -> integer bf16 in natural layout, to DRAM
                for j in range(O // 128):
                    wt = ldB.tile([128, F], FP32, tag="ld")
                    nc.sync.dma_start(wt[:, :], w[ts(j, 128), :])
                    wqn = stg.tile([128, F], BF16, tag="stg")
                    for c in range(C):
                        t1 = qtmp.tile([128, D], FP32, tag="qt")
                        nc.scalar.activation(
                            out=t1[:, :], in_=wt[:, ts(c, D)], func=AF.Copy,
                            bias=MAGIC, scale=inv5[:, C : C + 1],
                        )
                        nc.vector.tensor_scalar_add(
                            out=wqn[:, ts(c, D)], in0=t1[:, :], scalar1=-MAGIC,
                        )
                    nc.sync.dma_start(wq_nat[ts(j, 128), :], wqn[:, :])

            # ---------------- Phase C: matmul ----------------
            with (
                tc.tile_pool(name="wqt", bufs=2) as wqtp,
                tc.tile_pool(name="brhs", bufs=2) as brhsp,
                tc.tile_pool(name="opsum", bufs=2, space="PSUM") as opsum,
                tc.tile_pool(name="osb", bufs=3) as osbp,
            ):
                for p in range(NP):
                    wqT = wqtp.tile([128, KT, op_w], BF16, tag="wqt")
                    for k in range(KT):
                        nc.scalar.dma_start_transpose(
                            wqT[:, k, :], wq_nat[ds(p * op_w, op_w), ds(k * 128, 128)]
                        )
                    brhs = brhsp.tile([1, op_w], FP32, tag="brhs")
                    nc.sync.dma_start(brhs[:, :], b[0:1, ds(p * op_w, op_w)])
                    brhs_s = brhsp.tile([1, op_w], FP32, tag="brhs_s")
                    nc.vector.tensor_scalar(
                        out=brhs_s[:, :], in0=brhs[:, :],
                        scalar1=binv[0:1, 0:1], scalar2=None, op0=ALU.mult,
                    )
                    for tg in range(TT // grp):
                        psums = [
                            opsum.tile([128, op_w], FP32, tag=f"ps{i}", name=f"ps{i}")
                            for i in range(grp)
                        ]
                        # seed each tile's PSUM with bias/(sw*s0) (K=1 matmul)
                        for ti in range(grp):
                            nc.tensor.matmul(
                                psums[ti][:, :], lhsT=ones[:, :], rhs=brhs_s[:, :],
                                start=True, stop=False,
                            )
                        for c in range(C):
                            for ti in range(grp):
                                t = tg * grp + ti
                                for kk in range(KPC):
                                    k = c * KPC + kk
                                    nc.tensor.matmul(
                                        psums[ti][:, :],
                                        lhsT=xqT[:, k, ts(t, 128)],
                                        rhs=wqT[:, k, :],
                                        start=False,
                                        stop=(c == C - 1 and kk == KPC - 1),
                                    )
                                if c < C - 1:
                                    nc.vector.tensor_scalar(
                                        out=psums[ti][:, :], in0=psums[ti][:, :],
                                        scalar1=rr[:, c : c + 1], scalar2=None,
                                        op0=ALU.mult,
                                    )
                        for ti in range(grp):
                            t = tg * grp + ti
                            ot = osbp.tile([128, op_w], FP32, tag="osb")
                            nc.scalar.activation(
                                out=ot[:, :], in_=psums[ti][:, :], func=AF.Copy,
                                scale=rr[:, C - 1 : C],
                            )
                            nc.sync.dma_start(out[ts(t, 128), ds(p * op_w, op_w)], ot[:, :])

    nc.compile()
    return nc


_NC_CACHE = {}


def _get_nc(key):
    if key not in _NC_CACHE:
        T, F, O, n_cores = key
        _NC_CACHE[key] = build_nc(T, F, O, n_cores)
    return _NC_CACHE[key]


_LAST_RESULTS = None  # test harness can read exec_time_ns / trace from here


def run_sharded(x2d, weight, bias, n_cores, **run_kwargs):
    """x2d: [TOK, F] fp32; returns [TOK, O] fp32."""
    global _LAST_RESULTS
    TOK, F = x2d.shape
    O = weight.shape[0]
    T = TOK // n_cores
    WSL = O // n_cores
    nc = _get_nc((T, F, O, n_cores))
    bias2d = np.ascontiguousarray(bias.reshape(1, O), dtype=np.float32)
    in_maps = []
    for c in range(n_cores):
        in_maps.append(
            {
                "x": np.ascontiguousarray(x2d[c * T : (c + 1) * T], dtype=np.float32),
                "w": np.ascontiguousarray(weight, dtype=np.float32),
                "wsl": np.ascontiguousarray(
                    weight[c * WSL : (c + 1) * WSL], dtype=np.float32
                ),
                "b": bias2d,
            }
        )
    res = run_bass_kernel_spmd(nc, in_maps, list(range(n_cores)), **run_kwargs)
    _LAST_RESULTS = res
    return np.concatenate([res.results[c]["out"] for c in range(n_cores)], axis=0)


def kernel(x, weight, bias):
    x = np.ascontiguousarray(np.asarray(x), dtype=np.float32)
    weight = np.ascontiguousarray(np.asarray(weight), dtype=np.float32)
    bias = np.ascontiguousarray(np.asarray(bias), dtype=np.float32)
    B, S, F = x.shape
    O = weight.shape[0]
    out2d = run_sharded(x.reshape(B * S, F), weight, bias, N_CORES)
    return out2d.reshape(B, S, O)
